# revision 26
# baseline (speedup 1.0000x reference)
"""Causal single-head attention (B=4, S=2048, d=1024) on 8 trn2 NeuronCores.

Sharding: core c -> batch c//2, query-parity c%2. Queries of one batch are
split by even/odd 128-row blocks (interleaved so causal work balances);
every core runs the IDENTICAL program -- the host gathers each core's query
rows into a dense x_qT input, and two per-core [128,512] additive masks
encode the causal boundary. Each core redundantly computes K and V for its
batch. The host passes x pre-transposed (d-major) and PRE-QUANTIZED:
fp8(e4m3) copies of x for the Q/K projections plus fp8(16*Wq), fp8(16*Wk)
weights (the 16x scale escapes e4m3's subnormal range; the resulting
256x score scale is folded into the exp). V keeps full fp32 precision.

Mixed precision (validated ~1.4e-2 rel-max vs 2e-2 budget):
  Q/K projections + QK^T scores: fp8 DoubleRow matmuls (K=256 per
  instruction, 2x PE rate). E (exp output), E^T transposes and AV run in
  bf16. V projection stays float32r; V lives in SBUF (no DRAM round trip).

Schedule notes:
  P1 runs Q, then K for all 4 chunks (needs only the 3MB of fp8 x), then
  V -- so the 8MB fp32 x streams on the sync queue during the ~40us of
  Q/K matmuls. Weight DMAs ride the scalar queue; every x tile is ONE
  batched 3D-AP descriptor (each trigger costs ~630ns of queue-engine
  time). The x32 chunk DMAs are issued on the sync stream after all fp8
  x; later chunks blocking the sync engine on tile-slot reuse is
  harmless (nothing else queued there).
  P2 is i-major: each query block accumulates AV over all its key blocks
  in per-bank PSUM tile pairs (no SBUF output accumulator), the PE
  stream is software-pipelined one key-block ahead, causal mask adds
  touch only a 256-wide slice, and each output half finalizes with one
  ScalarE copy scaled by 1/l straight out of PSUM, then DMAs out on the
  sync HWDGE queue (faster end-of-kernel drain than SWDGE).

(Tried and rejected: pairwise AllGather K/V dedup across core pairs --
the NRT collective path costs ~18us per op serialized, far exceeding
the 42us of PE work it saves.)
"""

import os
import sys

import numpy as np

if "/opt/trn_rl_repo" not in sys.path:
    sys.path.insert(0, "/opt/trn_rl_repo")

# Enable the min-pop semaphore allocator (gated behind the RDH env vars):
# aggressive semaphore reuse shrinks the ~250-instruction per-semaphore
# clear cascade in the end-of-kernel teardown. No collectives or nested
# hardware loops in this kernel, so the known min-pop caveats don't apply.
os.environ.setdefault("TRNINF_ENABLE_CUSTOMCOMMS_RDH_AG", "1")

B = 4
S = 2048
D = 1024
NB = 8  # query blocks of 128 per core
KH = 8  # 128-row tiles along d_in / d_out
NEG = -1.0e9
WS = 16.0  # host-side weight scale for fp8 (scores come out 256x)
SCALE = float(D) ** -0.5 / (WS * WS)  # exp scale absorbs the 256x
_CACHE = {}
LAST_RESULT = None


def _build_nc():
    import contextlib

    import concourse.bacc as bacc
    import concourse.mybir as mybir
    import concourse.tile as tile

    F32 = mybir.dt.float32
    F32R = mybir.dt.float32r
    F8 = mybir.dt.float8e4
    BF = mybir.dt.bfloat16
    DR = mybir.MatmulPerfMode.DoubleRow

    nc = bacc.Bacc(None, target_bir_lowering=False)

    x_T = nc.dram_tensor("x_T", [D, S], F32, kind="ExternalInput")
    x_T8 = nc.dram_tensor("x_T8", [D, S], F8, kind="ExternalInput")
    x_qT8 = nc.dram_tensor("x_qT8", [D, NB * 128], F8, kind="ExternalInput")
    wq8 = nc.dram_tensor("wq8", [D, D], F8, kind="ExternalInput")
    wk8 = nc.dram_tensor("wk8", [D, D], F8, kind="ExternalInput")
    wv = nc.dram_tensor("wv", [D, D], F32, kind="ExternalInput")
    mask = nc.dram_tensor("mask", [2, 128, 512], F32, kind="ExternalInput")
    ident_in = nc.dram_tensor("ident", [128, 128], BF, kind="ExternalInput")
    y = nc.dram_tensor("y", [NB * 128, D], F32, kind="ExternalOutput")
    warm_dram = nc.dram_tensor("warm_scratch", [128, 256], F32)  # HAM warm-up

    # DRAM views with the 128-partition tiling of the d_in axis
    wq_t = wq8.rearrange("(kh p) n -> p kh n", p=128)
    wk_t = wk8.rearrange("(kh p) n -> p kh n", p=128)
    wv_t = wv.rearrange("(kh p) n -> p kh n", p=128)

    with tile.TileContext(nc) as tc:
        with contextlib.ExitStack() as ctx:
            persist = ctx.enter_context(tc.tile_pool(name="persist", bufs=1))

            ident = persist.tile([128, 128], BF)
            mask_sb = persist.tile([128, 2, 512], F32)
            q_T = persist.tile([128, KH, NB * 128], F8)  # [d_lo, d_hi, sq]
            k_T = persist.tile([128, KH, S], F8)  # [d_lo, d_hi, sk]
            v_keep = persist.tile([128, S // 128, D], BF)  # [s_lo, s_hi, e]
            l_acc = persist.tile([128, NB], F32)

            xT_view = x_T.rearrange("(kh p) s -> p kh s", p=128)
            xT8_view = x_T8.rearrange("(kh p) s -> p kh s", p=128)
            xqT8_view = x_qT8.rearrange("(kh p) s -> p kh s", p=128)

            # ---------------- Phase 1: projections ----------------
            with (
                tc.tile_pool(name="w8pool", bufs=2) as w8pool,
                tc.tile_pool(name="wvpool", bufs=1) as wvpool,
                tc.tile_pool(name="xq8", bufs=2) as xq8_pool,
                tc.tile_pool(name="x8", bufs=4) as x8_pool,
                tc.tile_pool(name="x32", bufs=2) as x32_pool,
                tc.tile_pool(name="mmps", bufs=8, space="PSUM") as mmps_pool,
            ):
                # PE warm-up: dependency-free fp32 matmuls on memset data
                # cover the first-DMA dead window and bring the HAM clock
                # gate to full rate. Written out so the chain isn't dead.
                warm = persist.tile([128, 256], F32)
                nc.vector.memset(warm, 0.0)
                wps = mmps_pool.tile([128, 512], F32, tag="mm")
                for m in range(9):
                    nc.tensor.matmul(
                        wps[:, :256], warm[:, :128], warm,
                        start=(m == 0), stop=(m == 8),
                    )
                nc.vector.tensor_copy(out=warm, in_=wps[:, :256])
                nc.gpsimd.dma_start(out=warm_dram[:, :], in_=warm)

                # Weight DMAs on the scalar (ACT) HWDGE queue; all x DMAs on
                # the sync (SP) queue (fp8 x first, then the four fp32
                # chunks -- later ones may block the sync engine on buffer
                # reuse, which is harmless).
                # Few LARGE descriptors: each DMA trigger costs ~630ns of
                # queue-engine issue time, so batch per-tile (3D APs) --
                # only the first weight halves stay split for Q startup.
                wq_sb = w8pool.tile([128, KH, D], F8, tag="w8")
                for hh in range(2):
                    nc.scalar.dma_start(
                        out=wq_sb[:, hh * 4 : (hh + 1) * 4, :],
                        in_=wq_t[:, hh * 4 : (hh + 1) * 4, :],
                    )
                wk_sb = w8pool.tile([128, KH, D], F8, tag="w8")
                nc.scalar.dma_start(out=wk_sb, in_=wk_t[:, :, :])
                wv_sb = wvpool.tile([128, KH, D], F32R, tag="wv")
                for hh in range(2):
                    nc.scalar.dma_start(
                        out=wv_sb[:, hh * 4 : (hh + 1) * 4, :],
                        in_=wv_t[:, hh * 4 : (hh + 1) * 4, :].bitcast(F32R),
                    )

                xq_tiles = []
                for strip in range(2):
                    xTq = xq8_pool.tile([128, KH, 512], F8, tag="xq8")
                    nc.sync.dma_start(
                        out=xTq,
                        in_=xqT8_view[:, :, strip * 512 : (strip + 1) * 512],
                    )
                    xq_tiles.append(xTq)
                x8_tiles = []
                for chunk in range(4):
                    xT8t = x8_pool.tile([128, KH, 512], F8, tag="x8")
                    nc.sync.dma_start(
                        out=xT8t,
                        in_=xT8_view[:, :, chunk * 512 : (chunk + 1) * 512],
                    )
                    x8_tiles.append(xT8t)
                x32_tiles = []
                for chunk in range(4):
                    xT32t = x32_pool.tile([128, KH, 512], F32R, tag="x32")
                    nc.sync.dma_start(
                        out=xT32t,
                        in_=xT_view[
                            :, :, chunk * 512 : (chunk + 1) * 512
                        ].bitcast(F32R),
                    )
                    x32_tiles.append(xT32t)

                def q_segment(strip, k_outer=False):
                    xTq = xq_tiles[strip]
                    if k_outer:
                        # startup: one PSUM bank per h-group so each arriving
                        # k-pair of wq/xTq immediately feeds 8 matmuls
                        qpss = []
                        for _h in range(KH):
                            qt = mmps_pool.tile([128, 512], F32, tag="mm")
                            qpss.append(qt)
                        for kp in range(KH // 2):
                            for h in range(KH):
                                nc.tensor.matmul(
                                    qpss[h],
                                    wq_sb[:, 2 * kp : 2 * kp + 2, h * 128 : (h + 1) * 128],
                                    xTq[:, 2 * kp : 2 * kp + 2, :],
                                    start=(kp == 0),
                                    stop=(kp == KH // 2 - 1),
                                    perf_mode=DR,
                                )
                        for h in range(KH):
                            nc.vector.tensor_copy(
                                out=q_T[:, h, strip * 512 : (strip + 1) * 512],
                                in_=qpss[h],
                            )
                        return
                    for h in range(KH):
                        qps = mmps_pool.tile([128, 512], F32, tag="mm")
                        for kp in range(KH // 2):
                            nc.tensor.matmul(
                                qps,
                                wq_sb[:, 2 * kp : 2 * kp + 2, h * 128 : (h + 1) * 128],
                                xTq[:, 2 * kp : 2 * kp + 2, :],
                                start=(kp == 0),
                                stop=(kp == KH // 2 - 1),
                                perf_mode=DR,
                            )
                        nc.vector.tensor_copy(
                            out=q_T[:, h, strip * 512 : (strip + 1) * 512],
                            in_=qps,
                        )

                def k_segment(chunk):
                    xT8t = x8_tiles[chunk]
                    for h in range(KH):
                        kps = mmps_pool.tile([128, 512], F32, tag="mm")
                        for kp in range(KH // 2):
                            nc.tensor.matmul(
                                kps,
                                wk_sb[:, 2 * kp : 2 * kp + 2, h * 128 : (h + 1) * 128],
                                xT8t[:, 2 * kp : 2 * kp + 2, :],
                                start=(kp == 0),
                                stop=(kp == KH // 2 - 1),
                                perf_mode=DR,
                            )
                        nc.vector.tensor_copy(
                            out=k_T[:, h, chunk * 512 : (chunk + 1) * 512], in_=kps
                        )

                def v_segment(chunk):
                    xT32t = x32_tiles[chunk]
                    for t in range(4):
                        for dh in range(2):
                            vps = mmps_pool.tile([128, 512], F32, tag="mm")
                            for k in range(KH):
                                nc.tensor.matmul(
                                    vps,
                                    xT32t[:, k, t * 128 : (t + 1) * 128],
                                    wv_sb[:, k, dh * 512 : (dh + 1) * 512],
                                    start=(k == 0),
                                    stop=(k == KH - 1),
                                )
                            nc.scalar.copy(
                                out=v_keep[
                                    :, chunk * 4 + t, dh * 512 : (dh + 1) * 512
                                ],
                                in_=vps,
                            )

                q_segment(0, k_outer=True)
                q_segment(1)
                for c in range(4):
                    k_segment(c)
                for c in range(4):
                    v_segment(c)

            # ---------------- Phase 2: attention ----------------
            # i-major: each query block i accumulates AV over all its key
            # blocks j=0..i//2 in ONE long PSUM group (no SBUF out_acc at
            # all); the finalize is a single fused (avps * 1/l) PSUM->SBUF
            # op. The PE stream is software-pipelined one j ahead: scores
            # for j+1 are emitted before transposes/AV of j, so the PE
            # never waits on the scalar exp except at the very tail.
            with (
                tc.tile_pool(name="esb", bufs=3) as esb_pool,
                tc.tile_pool(name="etsb", bufs=3) as etsb_pool,
                tc.tile_pool(name="lsb", bufs=4) as lsb_pool,
                tc.tile_pool(name="ysb", bufs=2) as ysb_pool,
                tc.tile_pool(name="sps", bufs=2, space="PSUM") as sps_pool,
                tc.tile_pool(name="etps", bufs=2, space="PSUM") as etps_pool,
                tc.tile_pool(name="avps", bufs=4, space="PSUM") as avps_pool,
            ):
                nc.sync.dma_start(out=ident, in_=ident_in[:, :])
                nc.sync.dma_start(out=mask_sb, in_=mask.rearrange("m p n -> p m n"))

                def emit_scores(i, j, diag, ncols):
                    sps = sps_pool.tile([128, 512], F32, tag="s")
                    for kp in range(KH // 2):
                        nc.tensor.matmul(
                            sps[:, :ncols],
                            q_T[:, 2 * kp : 2 * kp + 2, i * 128 : (i + 1) * 128],
                            k_T[:, 2 * kp : 2 * kp + 2, j * 512 : j * 512 + ncols],
                            start=(kp == 0),
                            stop=(kp == KH // 2 - 1),
                            perf_mode=DR,
                        )
                    if diag:
                        # the causal boundary only touches a 256-wide slice:
                        # even i -> cols [0:256) of m0; odd i -> [256:512)
                        # of m1 (cols [0:256) are always fully visible)
                        lo = 0 if i % 2 == 0 else 256
                        nc.vector.tensor_add(
                            out=sps[:, lo : lo + 256],
                            in0=sps[:, lo : lo + 256],
                            in1=mask_sb[:, i % 2, lo : lo + 256],
                        )
                    e_sb = esb_pool.tile([128, 512], BF, tag="e")
                    lpart = lsb_pool.tile([128, 1], F32, tag="l")
                    nc.scalar.activation(
                        out=e_sb[:, :ncols],
                        in_=sps[:, :ncols],
                        func=mybir.ActivationFunctionType.Exp,
                        scale=SCALE,
                        accum_out=lpart,
                    )
                    if j == 0:
                        nc.vector.tensor_copy(out=l_acc[:, i : i + 1], in_=lpart)
                    else:
                        nc.vector.tensor_add(
                            out=l_acc[:, i : i + 1],
                            in0=l_acc[:, i : i + 1],
                            in1=lpart,
                        )
                    return e_sb

                def emit_av(i, j, diag, ncols, e_sb, av, jmax):
                    njj = ncols // 128
                    etp = etps_pool.tile([128, 1024], BF, tag="et")
                    for jj in range(njj):
                        nc.tensor.transpose(
                            etp[:, jj * 128 : (jj + 1) * 128],
                            e_sb[:, jj * 128 : (jj + 1) * 128],
                            ident,
                        )
                    et = etsb_pool.tile([128, 512], BF, tag="ets")
                    nc.vector.tensor_copy(out=et[:, :ncols], in_=etp[:, :ncols])
                    # dh-major with separate per-bank PSUM tiles: each half
                    # finishes accumulating independently so the finalize
                    # of half 0 overlaps the PE work on half 1
                    for dh in range(2):
                        for jj in range(njj):
                            nc.tensor.matmul(
                                av[dh],
                                et[:, jj * 128 : (jj + 1) * 128],
                                v_keep[:, 4 * j + jj, dh * 512 : (dh + 1) * 512],
                                start=(j == 0 and jj == 0),
                                stop=(j == jmax and jj == njj - 1),
                                skip_group_check=True,
                            )

                def finalize_i(i, av, rinv):
                    ystage = ysb_pool.tile([128, D], F32, tag="y")
                    for dh in range(2):
                        # finalize on ScalarE: out = avps * (1/l), per half
                        nc.scalar.mul(
                            out=ystage[:, dh * 512 : (dh + 1) * 512],
                            in_=av[dh],
                            mul=rinv,
                        )
                        # sync HWDGE: faster end-of-kernel drain than SWDGE
                        nc.sync.dma_start(
                            out=y[i * 128 : (i + 1) * 128, dh * 512 : (dh + 1) * 512],
                            in_=ystage[:, dh * 512 : (dh + 1) * 512],
                        )

                # global one-ahead pipeline ACROSS i boundaries: the scores
                # of the next (i, j) stage are always emitted before the
                # transposes+AV of the previous stage, so the PE never
                # waits on the scalar exp -- even through single-j blocks
                order = (0, 1, 2, 3, 4, 5, 7, 6)  # 6 last: shortest tail
                stages = [
                    (i, j, i // 2) for i in order for j in range(i // 2 + 1)
                ]
                avs = {}
                rinvs = {}
                pend = None  # (i, j, diag, ncols, e_sb, jmax) awaiting AV
                for i, j, jmax in stages:
                    if j == 0:
                        av0 = avps_pool.tile([128, 512], F32, tag="av")
                        av1 = avps_pool.tile([128, 512], F32, tag="av")
                        avs[i] = [av0, av1]
                    diag = j == jmax
                    ncols = 256 if (diag and i % 2 == 0) else 512
                    e_sb = emit_scores(i, j, diag, ncols)
                    if diag:
                        # l complete once this lpart lands: 1/l while the
                        # PE finishes transposes + AV
                        rinv = lsb_pool.tile([128, 1], F32, tag="r")
                        nc.vector.reciprocal(out=rinv, in_=l_acc[:, i : i + 1])
                        rinvs[i] = rinv
                    if pend is not None:
                        pi, pj, pdiag, pncols, pe_sb, pjmax = pend
                        emit_av(pi, pj, pdiag, pncols, pe_sb, avs[pi], pjmax)
                        if pj == pjmax:
                            finalize_i(pi, avs[pi], rinvs[pi])
                    pend = (i, j, diag, ncols, e_sb, jmax)
                pi, pj, pdiag, pncols, pe_sb, pjmax = pend
                emit_av(pi, pj, pdiag, pncols, pe_sb, avs[pi], pjmax)
                finalize_i(pi, avs[pi], rinvs[pi])

    return nc


def _get_nc(finalize=True):
    key = "nc_fin" if finalize else "nc_raw"
    if key not in _CACHE:
        nc = _build_nc()
        if finalize:
            nc.finalize()
        _CACHE[key] = nc
    return _CACHE[key]


def make_in_maps(x, Wq, Wk, Wv):
    import ml_dtypes

    f8 = ml_dtypes.float8_e4m3
    bf = ml_dtypes.bfloat16
    ident = np.eye(128, dtype=np.float32).astype(bf)
    p = np.arange(128)[:, None]
    c = np.arange(512)[None, :]
    wq8 = np.ascontiguousarray(Wq * WS, dtype=np.float32).astype(f8)
    wk8 = np.ascontiguousarray(Wk * WS, dtype=np.float32).astype(f8)
    wv = np.ascontiguousarray(Wv, dtype=np.float32)
    in_maps = []
    for core in range(8):
        b, par = core // 2, core % 2
        # mask[0]: boundary block for even local i; mask[1]: odd local i
        m0 = np.where(c <= p + par * 128, 0.0, NEG).astype(np.float32)
        m1 = np.where(c <= 256 + par * 128 + p, 0.0, NEG).astype(np.float32)
        xb = np.asarray(x[b], dtype=np.float32)
        xb8 = xb.astype(f8)
        xq8 = xb8.reshape(16, 128, D)[par::2].reshape(NB * 128, D)
        in_maps.append(
            {
                "x_T": np.ascontiguousarray(xb.T),
                "x_T8": np.ascontiguousarray(xb8.T),
                "x_qT8": np.ascontiguousarray(xq8.T),
                "wq8": wq8,
                "wk8": wk8,
                "wv": wv,
                "mask": np.stack([m0, m1]),
                "ident": ident,
            }
        )
    return in_maps


def assemble_out(results):
    out = np.empty((B, S, D), dtype=np.float32)
    o4 = out.reshape(B, 16, 128, D)
    for core in range(8):
        b, par = core // 2, core % 2
        o4[b, par::2] = results[core]["y"].reshape(NB, 128, D)
    return out


def _ensure_axon_hooks_shim():
    """bass_utils imports antenv.axon_hooks when BASS_TRACE is set; provide a
    no-op fallback so a stray BASS_TRACE env var can't crash the run."""
    try:
        import antenv.axon_hooks  # noqa: F401
    except ImportError:
        import types

        import antenv

        mod = types.ModuleType("antenv.axon_hooks")
        mod.get_axon_ntff_profile_hook = lambda: None
        mod.set_axon_ntff_profile_hook = lambda h: None
        sys.modules["antenv.axon_hooks"] = mod
        antenv.axon_hooks = mod


def kernel(x, Wq, Wk, Wv):
    global LAST_RESULT
    from concourse.bass_utils import run_bass_kernel_spmd

    _ensure_axon_hooks_shim()
    nc = _get_nc(finalize=True)
    in_maps = make_in_maps(x, Wq, Wk, Wv)
    res = run_bass_kernel_spmd(nc, in_maps, core_ids=list(range(8)))
    LAST_RESULT = res
    return assemble_out(res.results)
